# revision 18
# baseline (speedup 1.0000x reference)
"""AdaptiveECE on 8 Trainium2 NeuronCores — v4 (fine-grained tiles, clean
input-DMA queue).

Data-parallel over N=1,000,000 rows: each core streams its 125,000-row shard
of logits [N,128] through SBUF once (64MB/core; the ~358GB/s HBM-per-core
limit makes the input stream the ~181us floor) and reduces it to two
per-row scalars:

  - mt[r] = max_c x[r,c]       exact f32 (VectorE segmented reduce_max per
                               tile, ~138us total)
  - s[r]  = sum_c exp(x[r,c])  PE path for all but the last few t-groups:
        PE transposes each [128,128] f32 block into PSUM, ScalarE exps the
        transposed block to bf16 SBUF, PE contracts over partitions with a
        sliding one-hot stationary into per-block PSUM sums. The final
        taper t-groups + the 72-row tail instead use ScalarE exp (row-major)
        + DVE segmented reduce_sum, so the last tiles drain in a short
        ACT->DVE->DMA chain with no end-of-run PSUM evacuation.

Lessons from v2/v3 traces baked in here:
  * PE has a p-state ramp (full clock only after ~3us of continuous work),
    so PE must stay loaded — offloading mid-run t-groups to DVE made every
    remaining matmul slower. Only the end-of-run taper leaves the PE path.
  * The input x buffers are released when the tile's last consumer (the
    PE transposes) completes, and the next input DMA *issue* waits on that
    release in the in-order sync-engine queue. v2 used 4MB chunks: one
    slow chunk stalled the whole stream (~12us of gaps). v4 uses 1MB
    (2-t-group) tiles with a ~19-tile window so jitter is absorbed.
  * Output DMAs (mt / s2 / s) used to sit in the same sync queue with
    semaphore waits, blocking later input issues. They now issue from the
    otherwise-idle GpSimd queue (mt, s2) and the Scalar queue (s), leaving
    the sync queue input-only.

The host finishes with O(N) work as the sharding hint prescribes ("finish
ECE on one host"): conf = exp(mt)/s, accuracy = (logits[r, labels[r]] ==
mt[r]) — exact since mt is the bit-exact max — then the global sort,
equal-count bin edges, per-bin (count, conf_sum, acc_sum), and the ECE.

Layout: each partition line holds G=8 consecutive rows (4KB contiguous DMA
runs). mt column (t*G + j), partition p  <->  shard row t*G*128 + p*G + j.
DVE-path sums land in s2 with the same mt-like layout. PE-path sums come
out block-indexed: s_d[k, S*512 + m] = sum of row gt*1024 + p*8 + (4h +
m//128) with p = m%128, where B = S*128 + k = 2*i + h and gt = pe_tgs[i].
"""

import sys
import types
from contextlib import ExitStack

import numpy as np

import concourse.bass as bass
import concourse.tile as tile
from concourse import bacc, mybir
from concourse.bass_utils import run_bass_kernel_spmd
from concourse.masks import make_identity


def _ensure_ntff_hook():
    """bass_utils imports antenv.axon_hooks when tracing is requested; the
    agent image lacks that module. Recreate it (wired to the axon .so) so a
    stray BASS_TRACE=1 in the environment cannot crash the run."""
    try:
        import antenv.axon_hooks  # noqa: F401
        return
    except ImportError:
        pass
    try:
        import antenv
        import trn_agent_boot.trn_boot as tb

        mod = types.ModuleType("antenv.axon_hooks")
        holder = [None]
        mod.set_axon_ntff_profile_hook = lambda h: holder.__setitem__(0, h)
        mod.get_axon_ntff_profile_hook = lambda: holder[0]
        sys.modules["antenv.axon_hooks"] = mod
        antenv.axon_hooks = mod
        try:
            mod.set_axon_ntff_profile_hook(
                tb._ntff_profile_via_ctypes("/opt/axon/libaxon_pjrt.so")
            )
        except Exception:
            pass
    except Exception:
        pass


_ensure_ntff_hook()

N = 1_000_000
C = 128
NBINS = 15
NCORES = 8
ROWS = N // NCORES  # 125_000 per core
G = 8  # rows per partition line (4KB contiguous DMA runs)
GR = G * 128  # rows per t-group
TILE_T = 2  # t-groups per x tile / input dma_start (release granularity)
X_BUFS = 19  # in-flight x tiles (19 x 1MB = 19MB SBUF)
M_DELAY = 3  # t-groups between exp and its sum-matmuls (keeps PE unstalled)
TAPER1 = 3  # trailing 1-t-group tiles (short final chain)
DVE_TAIL_TILES = 6  # trailing tiles on the DVE sum path (PE finishes early)
MID_DVE = (15, 30, 45)  # body tile ordinals moved to the DVE sum path
MT_BATCH_T = 8  # t-groups of mt columns per output DMA

_CACHE: dict = {}
LAST_RESULT = None  # BassKernelResults of the most recent device run


def _schedule(rows: int):
    """Shared device/host schedule.

    Returns (tiles, pe_tgs): tiles is a list of [t0, nfull, has_tail, pe]
    (pe = whether the tile's t-groups take the PE sum path); pe_tgs is the
    ordered list of t-group ids on the PE path (defines PSUM block order).
    """
    tfull = rows // GR
    tail = rows - tfull * GR
    tiles = []
    t0 = 0
    first = 1  # 1-t-group warmup tile so compute starts ASAP
    while t0 < tfull:
        left = tfull - t0
        if first:
            n = 1
        elif left <= TAPER1:
            n = 1  # 1-t-group taper tiles: short final chain
        else:
            n = min(TILE_T, left - TAPER1)
        first = 0
        tiles.append([t0, n, False, True])
        t0 += n
    for i, tl in enumerate(tiles):
        if i + DVE_TAIL_TILES >= len(tiles) or (i + 1) in MID_DVE:
            tl[3] = False
    if tail:
        tiles.insert(1, [tfull, 0, True, False])

    pe_tgs = []
    for t0, nfull, has_tail, pe in tiles:
        if pe:
            pe_tgs.extend(range(t0, t0 + nfull))
    return tiles, pe_tgs


def _build(rows: int):
    tfull = rows // GR
    tail = rows - tfull * GR
    tail_p = tail // G
    assert tail % G == 0, (rows, tail)
    nt = tfull + (1 if tail else 0)
    tt = nt * G  # mt / s2 output columns

    tiles, pe_tgs = _schedule(rows)
    pe_index = {gt: i for i, gt in enumerate(pe_tgs)}
    nblk = 2 * len(pe_tgs)
    nsg = (nblk + 127) // 128

    nc = bacc.Bacc("TRN2", target_bir_lowering=False, debug=False)
    lg = nc.dram_tensor("logits", [rows, C], mybir.dt.float32, kind="ExternalInput").ap()
    s_d = nc.dram_tensor("s", [128, nsg * 512], mybir.dt.bfloat16, kind="ExternalOutput").ap()
    s2_d = nc.dram_tensor("s2", [128, tt], mybir.dt.bfloat16, kind="ExternalOutput").ap()
    mt_d = nc.dram_tensor("mt", [128, tt], mybir.dt.float32, kind="ExternalOutput").ap()

    # [p, t, (j c)] view: row t*1024 + p*8 + j; (j c) is 4KB-contiguous per (p,t)
    lg_t = (
        lg[0 : tfull * GR, :].rearrange("(t p j) c -> p t (j c)", p=128, j=G)
        if tfull
        else None
    )

    with tile.TileContext(nc) as tc, ExitStack() as ctx:
        singles = ctx.enter_context(tc.tile_pool(name="singles", bufs=1))
        xpool = ctx.enter_context(tc.tile_pool(name="x", bufs=X_BUFS))
        epool = ctx.enter_context(tc.tile_pool(name="e", bufs=3 + M_DELAY))
        dpool = ctx.enter_context(tc.tile_pool(name="d", bufs=2))
        tpsum = ctx.enter_context(tc.tile_pool(name="tp", bufs=3, space="PSUM"))
        spsum = ctx.enter_context(tc.tile_pool(name="sp", bufs=1, space="PSUM"))

        ident = singles.tile([128, 128], mybir.dt.float32)
        make_identity(nc, ident[:])
        # sliding one-hot stationary: onehot[:, 127-k : 255-k] has its 1 at col k
        onehot = singles.tile([128, 255], mybir.dt.bfloat16)
        nc.vector.memset(onehot[:], 0.0)
        nc.vector.memset(onehot[:, 127:128], 1.0)

        mt_sb = singles.tile([128, tt], mybir.dt.float32)
        s2_sb = singles.tile([128, tt], mybir.dt.bfloat16)
        s_sb = singles.tile([128, nsg * 512], mybir.dt.bfloat16)
        s_ps = [
            spsum.tile([128, 512], mybir.dt.float32, name=f"s_ps{i}")
            for i in range(nsg)
        ]
        s_count = [0] * nsg  # matmuls issued into each sum group
        s_total = [0] * nsg  # matmuls each group will receive
        s_done = [False] * nsg  # evacuation emitted
        for b in range(nblk):
            s_total[b // 128] += 1

        pending_m = []  # (et_tile, pe_idx) awaiting their sum-matmuls

        def flush_m(limit):
            while len(pending_m) > limit:
                et, pi = pending_m.pop(0)
                for h in (0, 1):
                    b = 2 * pi + h
                    sg = b // 128
                    k = b % 128
                    nc.tensor.matmul(
                        s_ps[sg][:],
                        onehot[:, 127 - k : 255 - k],
                        et[:, h * 512 : (h + 1) * 512],
                        start=(s_count[sg] == 0),
                        stop=(s_count[sg] == s_total[sg] - 1),
                        skip_group_check=True,
                    )
                    s_count[sg] += 1

        def evacuate_ready():
            # emit each sum group's PSUM->SBUF->DRAM drain as soon as its
            # last matmul has been queued, so it lands in the (in-order)
            # scalar queue ahead of later exps
            for sg in range(nsg):
                if not s_done[sg] and s_count[sg] == s_total[sg]:
                    s_done[sg] = True
                    nc.scalar.copy(
                        s_sb[:, sg * 512 : (sg + 1) * 512], s_ps[sg][:]
                    )
                    nc.scalar.dma_start(
                        s_d[:, sg * 512 : (sg + 1) * 512],
                        s_sb[:, sg * 512 : (sg + 1) * 512],
                    )

        mt_sent = 0  # mt columns already written out

        def send_mt(upto_cols, force=False):
            nonlocal mt_sent
            if upto_cols - mt_sent >= MT_BATCH_T * G or (force and upto_cols > mt_sent):
                nc.gpsimd.dma_start(
                    mt_d[:, mt_sent:upto_cols], mt_sb[:, mt_sent:upto_cols]
                )
                mt_sent = upto_cols

        s2_lo = None  # first column of pending (unsent) DVE-path sums

        pending_dve = []  # deferred (lo, hi, ed) reduce_sums

        def flush_dve():
            while pending_dve:
                lo_c, hi_c, ed = pending_dve.pop(0)
                with nc.allow_low_precision(
                    reason="bf16 softmax-denominator sums; ECE tolerance 2e-2"
                ):
                    nc.vector.reduce_sum(
                        s2_sb[:, lo_c:hi_c], ed[:],
                        axis=mybir.AxisListType.X,
                    )

        for t0, nfull, has_tail, pe in tiles:
            ntg = nfull + (1 if has_tail else 0)
            ncols = ntg * G
            x = xpool.tile([128, ncols, C], mybir.dt.float32)
            if nfull:
                nc.sync.dma_start(
                    x[:, 0 : nfull * G, :].rearrange(
                        "p a c -> p (a c)"
                    ).rearrange("p (t b) -> p t b", b=G * C),
                    lg_t[:, t0 : t0 + nfull, :],
                )
            if has_tail:
                nc.vector.memset(x[:, nfull * G :, :], 0.0)
                tail_src = lg[tfull * GR : rows, :].rearrange("(p j) c -> p (j c)", j=G)
                nc.sync.dma_start(
                    x[0:tail_p, nfull * G :, :].rearrange("p a c -> p (a c)"), tail_src
                )

            # exact row max on DVE; deferred reduce_sums slot in behind
            # it so they never block the next tile's max in the queue
            nc.vector.reduce_max(
                mt_sb[:, t0 * G : t0 * G + ncols], x[:],
                axis=mybir.AxisListType.X,
            )
            flush_dve()

            if pe:
                # PE path: transpose -> exp(PSUM) -> one-hot sum matmuls
                for lt in range(ntg):
                    pi = pe_index[t0 + lt]
                    tp = tpsum.tile([128, 1024], mybir.dt.float32)
                    for j in range(8):
                        nc.tensor.matmul(
                            tp[:, j * 128 : (j + 1) * 128],
                            x[:, lt * G + j, :],
                            ident[:],
                            is_transpose=True,
                            skip_group_check=True,
                        )
                    et = epool.tile([128, 1024], mybir.dt.bfloat16)
                    nc.scalar.activation(
                        et[:], tp[:], mybir.ActivationFunctionType.Exp
                    )
                    pending_m.append((et, pi))
                    flush_m(M_DELAY)
                evacuate_ready()
            else:
                # DVE path: exp(SBUF, row-major) -> segmented reduce_sum
                # (deferred one tile so it follows the next tile's max)
                ed = dpool.tile([128, ncols, C], mybir.dt.bfloat16)
                nc.scalar.activation(
                    ed[:], x[:], mybir.ActivationFunctionType.Exp
                )
                pending_dve.append((t0 * G, t0 * G + ncols, ed))
                if s2_lo is None:
                    s2_lo = t0 * G
                if has_tail:
                    # the early tail tile sits at the far end of s2 and is
                    # flushed alone
                    flush_dve()
                    nc.gpsimd.dma_start(
                        s2_d[:, s2_lo : t0 * G + ncols],
                        s2_sb[:, s2_lo : t0 * G + ncols],
                    )
                    s2_lo = None
                elif t0 * G + ncols == tfull * G:
                    flush_dve()
                    nc.gpsimd.dma_start(
                        s2_d[:, s2_lo : t0 * G + ncols],
                        s2_sb[:, s2_lo : t0 * G + ncols],
                    )
                    s2_lo = None

            if not has_tail:
                # (the tail tile runs early, out of column order; its mt
                # columns go out with the final force-send)
                upto = (t0 + nfull) * G
                send_mt(tt if t0 + nfull == tfull else upto)

        flush_m(0)
        evacuate_ready()
        assert all(s_done), (s_count, s_total)
        send_mt(tt, force=True)

    nc.compile()
    return nc


def _unpermute_mt(a_2d, rows):
    """Device mt/s2 [128, TT] -> per-row vector [rows].

    Column t*G+j, partition p <-> row t*G*128 + p*G + j.
    """
    tfull = rows // GR
    tail = rows - tfull * GR
    tail_p = tail // G
    out = np.empty(rows, a_2d.dtype)
    nmain = tfull * GR
    out[:nmain] = (
        a_2d[:, : tfull * G].reshape(128, tfull, G).transpose(1, 0, 2).reshape(-1)
    )
    if tail:
        out[nmain:] = a_2d[:tail_p, tfull * G :].reshape(-1)
    return out


def _merge_s(s_2d, s2_2d, rows):
    """Combine PE-path block sums and DVE-path mt-like sums -> [rows].

    s_2d[k, S*512 + m] = sum for PE block B = S*128 + k: with i = B//2,
    h = B%2, the block covers rows gt*1024 + p*8 + (4h + m//128) where
    gt = pe_tgs[i], p = m%128.  s2_2d has mt layout for DVE t-groups.
    """
    tiles, pe_tgs = _schedule(rows)
    tfull = rows // GR
    tail = rows - tfull * GR
    nt = tfull + (1 if tail else 0)
    nblk = 2 * len(pe_tgs)
    nsg = (nblk + 127) // 128

    out = np.empty((nt, 128, G), np.float32)  # [gt, p, j]

    # DVE-path t-groups from s2 (same permutation as mt)
    s2_rows = s2_2d.reshape(128, nt, G).transpose(1, 0, 2)  # [gt, p, j]
    pe_set = set(pe_tgs)
    for gt in range(nt):
        if gt not in pe_set:
            out[gt] = s2_rows[gt]

    # PE-path blocks
    blocks = (
        s_2d.reshape(128, nsg, 512).transpose(1, 0, 2).reshape(nsg * 128, 512)[:nblk]
    )
    # [B, m] -> [i, h, j', p]; j = 4h + j'
    b4 = blocks.reshape(len(pe_tgs), 2, 4, 128)
    for i, gt in enumerate(pe_tgs):
        out[gt] = b4[i].transpose(2, 0, 1).reshape(128, G)  # [p, (h j')]

    return out.reshape(-1)[:rows].copy()


def _finish(conf, acc):
    """Mirror of the reference ECE finishing on host."""
    n = conf.shape[0]
    sorted_conf = np.sort(conf)
    q = np.linspace(0.0, float(n), NBINS + 1, dtype=np.float32)
    edges = np.interp(q, np.arange(n, dtype=np.float32), sorted_conf).astype(np.float32)
    idx = np.searchsorted(edges[1:-1], conf, side="left")
    valid = (conf > edges[0]) & (conf <= edges[-1])
    idx = np.where(valid, idx, NBINS)
    cnt = np.bincount(idx, minlength=NBINS + 1)[:NBINS].astype(np.float32)
    csum = np.bincount(idx, weights=conf.astype(np.float64), minlength=NBINS + 1)[
        :NBINS
    ].astype(np.float32)
    asum = np.bincount(idx, weights=acc.astype(np.float64), minlength=NBINS + 1)[
        :NBINS
    ].astype(np.float32)
    prop = cnt / np.float32(n)
    safe = np.maximum(cnt, 1.0)
    gap = np.abs(csum / safe - asum / safe)
    ece = np.sum(np.where(cnt > 0, gap * prop, 0.0), dtype=np.float32)
    return np.asarray(ece, dtype=np.float32).reshape(1)


def kernel(logits, labels, trace: bool = False):
    global LAST_RESULT
    logits = np.asarray(logits)
    labels = np.asarray(labels)
    assert logits.shape == (N, C), logits.shape

    if "nc" not in _CACHE:
        _CACHE["nc"] = _build(ROWS)
    nc = _CACHE["nc"]

    in_maps = [
        {"logits": np.ascontiguousarray(logits[i * ROWS : (i + 1) * ROWS], np.float32)}
        for i in range(NCORES)
    ]
    res = run_bass_kernel_spmd(nc, in_maps, core_ids=list(range(NCORES)), trace=trace)
    LAST_RESULT = res

    s = np.empty(N, np.float32)
    mt = np.empty(N, np.float32)
    for i in range(NCORES):
        s[i * ROWS : (i + 1) * ROWS] = _merge_s(
            res.results[i]["s"].astype(np.float32),
            res.results[i]["s2"].astype(np.float32),
            ROWS,
        )
        mt[i * ROWS : (i + 1) * ROWS] = _unpermute_mt(res.results[i]["mt"], ROWS)

    # mt = exact per-row max (f32); accuracy = logit at the label equals it
    xlab = logits[np.arange(N), labels.astype(np.int64)]
    acc = (xlab == mt).astype(np.float32)
    conf = (np.exp(mt) / s).astype(np.float32)
    return _finish(conf, acc)
